# revision 5
# baseline (speedup 1.0000x reference)
"""Trainium2 Bass kernel for nn_EquivarianceNetwork (grouped 4-layer MLP).

Math (per sample b, TWO_N=16 groups, D=64):
  xr = x.reshape(B, 16, 64)
  scalars[b, n, m] = <xr[b,n], xr[b,m]>                  # [B, 256]
  per group l: h = tanh(...W0/W1/W2...), coeffs = h @ W3 + b3   # [B, 16]
  out[b, l*64:(l+1)*64] = sum_n coeffs[l,b,n] * xr[b,n]

Distribution: data-parallel over batch across 8 cores (weights replicated).
Per core B_local = 2048.

Engine plan per core (v2, fp16 + fp8-DoubleRow):
  - All activations/weights f16 except W1, which is split W1 = W1_hi + W1_lo
    (both fp8e4m3, x32 scaled) and consumed with MatmulPerfMode.DoubleRow:
    each DR pass contracts two k-slices at 0.5 cyc/row, and the two passes
    per k-slice pair are arranged as (hi[k0],lo[k1]) + (lo[k0],hi[k1]) so
    their PSUM sum is exactly h0 @ (W1_hi+W1_lo) -- full dual-fp8 weight
    precision; only h0's single-e4m3 quantization (~2.5% rms) remains, which
    measures ~9e-3 end-to-end rel err (gate 2e-2).  L1 thus runs 2x.
  - h0 is written as fp8e4 directly by the ACT tanh; h1/h2 as f16.
  - x is resident in SBUF as f16 (no per-(l,s) reloads for the finals).
  - Gram uses a packed 136-row triangle (f16 in/out, contiguous reduce) and
    L0 contracts K=136 (128+8) against host-folded W0.
  - DVE: Gram mults(small dl)+reduces, most finals; GPSIMD: Gram mults
    (large dl), rest of finals.
"""
import numpy as np
import ml_dtypes

import concourse.bass as bass
import concourse.mybir as mybir
import concourse.tile as tile
from concourse import bacc
from concourse.bass_utils import run_bass_kernel_spmd
from concourse.masks import make_identity

F32 = mybir.dt.float32
F16 = mybir.dt.float16
F8 = mybir.dt.float8e4
TANH = mybir.ActivationFunctionType.Tanh
DR = mybir.MatmulPerfMode.DoubleRow
E4M3 = ml_dtypes.float8_e4m3

N_CORES = 8
B = 16384
TWO_N = 16
D = 64
B_LOC = B // N_CORES          # 2048
N_SUB = B_LOC // 128          # 16 subtiles of 128 samples
N_BT = B_LOC // 512           # 4 batch tiles of 512 (matmul free dim)
H = 1024                      # hidden width
K_TRI = 136                   # packed upper-triangle scalar count
W1_SCALE = 32.0

# triangle row offsets: row u = TRI_OFF[dl] + n holds pair (n, n+dl)
TRI_OFF = [0]
for _dl in range(1, 17):
    TRI_OFF.append(TRI_OFF[-1] + (TWO_N - (_dl - 1)))


def _build_program():
    nc = bacc.Bacc()

    x = nc.declare_dram_parameter("x", [B_LOC, TWO_N * D], F16, isOutput=False)
    W0 = nc.declare_dram_parameter("W0", [TWO_N, K_TRI, H], F16, isOutput=False)
    W1 = nc.declare_dram_parameter("W1", [TWO_N, 2, 128, 8192], F8,
                                   isOutput=False)
    W2 = nc.declare_dram_parameter("W2", [TWO_N, H, H], F16, isOutput=False)
    W3 = nc.declare_dram_parameter("W3", [TWO_N, H, TWO_N], F16,
                                   isOutput=False)
    b0 = nc.declare_dram_parameter("b0", [TWO_N, H], F32, isOutput=False)
    b1 = nc.declare_dram_parameter("b1", [TWO_N, H], F32, isOutput=False)
    b2 = nc.declare_dram_parameter("b2", [TWO_N, H], F32, isOutput=False)
    b3 = nc.declare_dram_parameter("b3", [TWO_N, TWO_N], F32, isOutput=False)
    y = nc.declare_dram_parameter("y", [B_LOC, TWO_N * D], F32, isOutput=True)

    with tile.TileContext(nc) as tc:
        with tc.tile_pool(name="res", bufs=1) as res, \
             tc.tile_pool(name="work", bufs=2) as wk, \
             tc.tile_pool(name="w0p", bufs=2) as w0p, \
             tc.tile_pool(name="w1p", bufs=3) as w1p, \
             tc.tile_pool(name="w2p", bufs=3) as w2p, \
             tc.tile_pool(name="w3p", bufs=2) as w3p, \
             tc.tile_pool(name="hp", bufs=2) as hp, \
             tc.tile_pool(name="fin", bufs=4) as finp, \
             tc.tile_pool(name="ps", bufs=4, space="PSUM") as ps:

            # resident x (f16), 16 subtiles of 128 samples
            x16 = [res.tile([128, TWO_N * D], F16, name=f"x16_{s}")
                   for s in range(N_SUB)]
            for s in range(N_SUB):
                nc.sync.dma_start(out=x16[s], in_=x[128 * s:128 * (s + 1), :])

            ident = res.tile([128, 128], F32)
            make_identity(nc, ident)
            ident16 = res.tile([128, 128], F16)
            make_identity(nc, ident16)

            # resident: transposed packed scalars [136, B_LOC] f16
            scalT = res.tile([128, B_LOC], F16, name="scalT")
            scalT2 = res.tile([8, B_LOC], F16, name="scalT2")
            # resident: coeffs batch-major per subtile [128, 256] f16
            coeff = [res.tile([128, 256], F16, name=f"coeff{s}")
                     for s in range(N_SUB)]

            # ---------------- Gram for one subtile of 128 samples ----------
            # prod[p, u*64+d] = x[p, n*64+d]*x[p, (n+dl)*64+d], u=TRI_OFF[dl]+n
            # then two contiguous reduces (DVE, f16 2x) -> sbm[p, 0:136].
            def gram(s):
                xg = x16[s]
                prod = wk.tile([128, K_TRI * D], F16, name="prod", tag="prod")
                sbm = wk.tile([128, K_TRI], F16, name="sbm", tag="sbm")
                for dl in range(TWO_N):
                    npair = TWO_N - dl
                    meng = nc.vector if dl < 5 else nc.gpsimd
                    off = TRI_OFF[dl] * D
                    meng.tensor_mul(
                        prod[:, off:off + npair * D],
                        xg[:, 0:npair * D],
                        xg[:, dl * D:(dl + npair) * D],
                    )
                split = TRI_OFF[5]  # rows 0:70 from DVE mults, rest GPSIMD
                with nc.allow_low_precision(
                        reason="f16 gram reduce; DVE accumulates fp32 "
                               "internally, only the store is f16"):
                    nc.vector.tensor_reduce(
                        sbm[:, 0:split],
                        prod[:, 0:split * D].rearrange("p (n d) -> p n d",
                                                       d=D),
                        axis=mybir.AxisListType.X, op=mybir.AluOpType.add)
                    nc.vector.tensor_reduce(
                        sbm[:, split:K_TRI],
                        prod[:, split * D:].rearrange("p (n d) -> p n d",
                                                      d=D),
                        axis=mybir.AxisListType.X, op=mybir.AluOpType.add)
                pt = ps.tile([128, 128], F16, name="tp", tag="tp", bufs=2)
                nc.tensor.transpose(pt[:, :], sbm[:, 0:128], ident16)
                nc.scalar.copy(scalT[:, 128 * s:128 * (s + 1)], pt[:, :])
                pt2 = ps.tile([8, 128], F16, name="tp2", tag="tp", bufs=2)
                nc.tensor.transpose(pt2[:, :], sbm[:, 128:K_TRI], ident16)
                nc.scalar.copy(scalT2[:, 128 * s:128 * (s + 1)], pt2[:, :])

            # ---- biases: preload all groups once, transposed on PE ----
            # b012_all[p, li, ot, l] = b_li[l, ot*128 + p]
            b012_all = res.tile([128, 3, 8, TWO_N], F32)
            b3_all = res.tile([16, TWO_N], F32)   # [n, l]

            def bias_preload():
                for li, bsrc in enumerate((b0, b1, b2)):
                    bnat = wk.tile([TWO_N, H], F32, name=f"bnat{li}",
                                   tag="bnat")
                    nc.sync.dma_start(out=bnat, in_=bsrc[:, :])
                    for ot in range(8):
                        pt = ps.tile([128, 128], F32, name="tpb", tag="tp",
                                     bufs=2)
                        nc.tensor.transpose(
                            pt[:, 0:TWO_N], bnat[:, 128 * ot:128 * (ot + 1)],
                            ident[0:TWO_N, 0:TWO_N])
                        nc.scalar.copy(b012_all[:, li, ot, :], pt[:, 0:TWO_N])
                b3nat = wk.tile([TWO_N, TWO_N], F32, name="b3nat", tag="bnat")
                nc.sync.dma_start(out=b3nat, in_=b3[:, :])
                pt = ps.tile([128, 128], F32, name="tpb3", tag="tp", bufs=2)
                nc.tensor.transpose(pt[0:TWO_N, 0:TWO_N], b3nat[:, :],
                                    ident[0:TWO_N, 0:TWO_N])
                nc.scalar.copy(b3_all[:, :], pt[0:TWO_N, 0:TWO_N])

            # ---- final contraction for one (l, subtile):
            # y[bsub, l*64+d] = sum_n coeff[b, 16l+n] * x[b, 64n+d]
            def final_unit(l, s):
                xg = x16[s]
                prod = wk.tile([128, TWO_N * D], F16, name="prod2", tag="fpr")
                c = coeff[s]
                in1 = bass.AP(tensor=c.tensor, offset=c.offset + 16 * l,
                              ap=[c.ap[0], [1, TWO_N], [0, D]])
                meng = (nc.vector if s % 2 == 0 else nc.gpsimd) \
                    if l == TWO_N - 1 else \
                    (nc.vector if s % 8 < 5 else nc.gpsimd)
                meng.tensor_mul(
                    prod[:, :].rearrange("p (n d) -> p n d", d=D),
                    xg[:, :].rearrange("p (n d) -> p n d", d=D),
                    in1)
                meng.tensor_add(prod[:, 0:512], prod[:, 0:512],
                                prod[:, 512:1024])
                meng.tensor_add(prod[:, 0:256], prod[:, 0:256],
                                prod[:, 256:512])
                meng.tensor_add(prod[:, 0:128], prod[:, 0:128],
                                prod[:, 128:256])
                fcol = finp.tile([128, D], F32, name="fcol", tag="fcol")
                meng.tensor_add(fcol[:, :], prod[:, 0:D], prod[:, D:2 * D])
                nc.sync.dma_start(
                    out=y[128 * s:128 * (s + 1), D * l:D * (l + 1)],
                    in_=fcol[:, :])

            # ---------------- Phase B: grouped MLP ----------------
            for s in range(4):
                gram(s)
            bias_preload()

            for l in range(TWO_N):
                w0a = w0p.tile([128, H], F16, name="w0a", tag="w0a")
                nc.sync.dma_start(out=w0a, in_=W0[l, 0:128, :])
                w0b = w0p.tile([8, H], F16, name="w0b", tag="w0b")
                nc.sync.dma_start(out=w0b, in_=W0[l, 128:K_TRI, :])
                # W1 dual-fp8 DoubleRow-packed: per half
                # [128, jj(2), pass(2), pair(2), out(1024)] fp8
                w1h = []
                for half in range(2):
                    wt = w1p.tile([128, 8192], F8, name=f"w1_{half}",
                                  tag="w1")
                    nc.sync.dma_start(out=wt, in_=W1[l, half, :, :])
                    w1h.append(wt)
                w2h = []
                for half in range(2):
                    wt = w2p.tile([128, 4, H], F16, name=f"w2_{half}",
                                  tag="w2")
                    nc.sync.dma_start(
                        out=wt,
                        in_=W2[l, 512 * half:512 * (half + 1), :]
                        .rearrange("(t p) m -> p t m", p=128))
                    w2h.append(wt)
                w3t = w3p.tile([128, 8, TWO_N], F16, name="w3t", tag="w3")
                nc.sync.dma_start(
                    out=w3t,
                    in_=W3[l, :, :].rearrange("(t p) m -> p t m", p=128))

                for bt in range(N_BT):
                    bs = 512 * bt
                    # L0: scalT (K=136) -> h0 (fp8e4 for the DR L1)
                    h0 = hp.tile([128, 8, 512], F8, name="h0", tag="h0")
                    for ot in range(8):
                        pt = ps.tile([128, 512], F32, name="mlp", tag="mlp",
                                     bufs=5)
                        nc.tensor.matmul(
                            pt[:, :], w0a[:, 128 * ot:128 * (ot + 1)],
                            scalT[:, bs:bs + 512], start=True, stop=False)
                        nc.tensor.matmul(
                            pt[:, :], w0b[:, 128 * ot:128 * (ot + 1)],
                            scalT2[:, bs:bs + 512], start=False, stop=True)
                        nc.scalar.activation(
                            h0[:, ot, :], pt[:, :], TANH,
                            bias=b012_all[:, 0, ot, l:l + 1])
                    # L1: fp8 DoubleRow, W1 = hi+lo exactly
                    h1 = hp.tile([128, 8, 512], F16, name="h1", tag="h1")
                    for ot in range(8):
                        pt = ps.tile([128, 512], F32, name="mlp", tag="mlp",
                                     bufs=5)
                        for j in range(4):          # k-slice pairs (2j, 2j+1)
                            half, jj = j // 2, j % 2
                            wt = w1h[half]
                            rhs = bass.AP(
                                tensor=h0.tensor,
                                offset=h0.offset + 1024 * j,
                                ap=[h0.ap[0], [512, 2], [1, 512]])
                            for p in range(2):
                                lhsT = bass.AP(
                                    tensor=wt.tensor,
                                    offset=(wt.offset + 4096 * jj + 2048 * p
                                            + 128 * ot),
                                    ap=[wt.ap[0], [1024, 2], [1, 128]])
                                nc.tensor.matmul(
                                    pt[:, :], lhsT, rhs,
                                    start=(j == 0 and p == 0),
                                    stop=(j == 3 and p == 1),
                                    perf_mode=DR)
                        nc.scalar.activation(
                            h1[:, ot, :], pt[:, :], TANH,
                            scale=1.0 / W1_SCALE,
                            bias=b012_all[:, 1, ot, l:l + 1])
                    # L2: f16
                    h2 = hp.tile([128, 8, 512], F16, name="h2", tag="h2")
                    for ot in range(8):
                        pt = ps.tile([128, 512], F32, name="mlp", tag="mlp",
                                     bufs=5)
                        for kt in range(8):
                            nc.tensor.matmul(
                                pt[:, :],
                                w2h[kt // 4][:, kt % 4,
                                             128 * ot:128 * (ot + 1)],
                                h1[:, kt, :],
                                start=(kt == 0), stop=(kt == 7))
                        nc.scalar.activation(
                            h2[:, ot, :], pt[:, :], TANH,
                            bias=b012_all[:, 2, ot, l:l + 1])
                    # L3 -> coeffs [16, 512] + bias, transpose to batch-major
                    p3 = ps.tile([16, 512], F32, name="p3", tag="p3", bufs=1)
                    for kt in range(8):
                        nc.tensor.matmul(p3[:, :], w3t[:, kt, :],
                                         h2[:, kt, :],
                                         start=(kt == 0), stop=(kt == 7))
                    csb = wk.tile([16, 512], F16, name="csb", tag="csb")
                    nc.scalar.add(csb[:, :], p3[:, :], b3_all[:, l:l + 1])
                    for jj in range(4):
                        tpc = ps.tile([128, 16], F16, name="tpc", tag="tp",
                                      bufs=2)
                        nc.tensor.transpose(
                            tpc[:, 0:16], csb[:, 128 * jj:128 * (jj + 1)],
                            ident16[0:16, 0:16])
                        sub = 4 * bt + jj
                        nc.scalar.copy(
                            coeff[sub][:, 16 * l:16 * (l + 1)], tpc[:, 0:16])

                    if l == 0:
                        # l=0 is Gram-bound: emit the next Gram group here
                        # and defer finals to the end of the group loop.
                        if bt < 3:
                            for s in range(4 * bt + 4, 4 * bt + 8):
                                gram(s)
                    else:
                        for s in range(4 * bt, 4 * bt + 4):
                            final_unit(l, s)

                if l == 0:
                    for s in range(N_SUB):
                        final_unit(l, s)

    nc.finalize()
    return nc


_NC = None


def build_in_maps(x, W0, b0, W1, b1, W2, b2, W3, b3):
    x16 = np.ascontiguousarray(np.asarray(x, dtype=np.float32)
                               .astype(np.float16))
    # W0 folded over symmetric scalar pairs into the packed 136-row triangle
    # (row u = TRI_OFF[dl]+n holds pair (n, m=n+dl); off-diagonal rows get
    # W0[nm] + W0[mn] since scal(n,m)==scal(m,n)).
    W0v = np.asarray(W0, np.float32).reshape(TWO_N, TWO_N, TWO_N, H)
    rows = []
    for dl in range(TWO_N):
        for n in range(TWO_N - dl):
            m = n + dl
            r = W0v[:, n, m, :]
            if dl > 0:
                r = r + W0v[:, m, n, :]
            rows.append(r)
    W0f = np.ascontiguousarray(
        np.stack(rows, axis=1).astype(np.float16))          # [16, 136, 1024]

    # W1 scaled x32, split hi/lo in e4m3, packed for DoubleRow:
    # [l, half, part, jj, pass, pair, out] with pass sums giving hi+lo
    W1s = np.asarray(W1, np.float32) * W1_SCALE
    hi = W1s.astype(E4M3)
    lo = (W1s - hi.astype(np.float32)).astype(E4M3)
    hi4 = hi.reshape(TWO_N, 8, 128, H)
    lo4 = lo.reshape(TWO_N, 8, 128, H)
    W1dr = np.zeros((TWO_N, 2, 128, 2, 2, 2, H), E4M3)
    for half in range(2):
        for jj in range(2):
            k0 = 4 * half + 2 * jj
            k1 = k0 + 1
            W1dr[:, half, :, jj, 0, 0, :] = hi4[:, k0]
            W1dr[:, half, :, jj, 0, 1, :] = lo4[:, k1]
            W1dr[:, half, :, jj, 1, 0, :] = lo4[:, k0]
            W1dr[:, half, :, jj, 1, 1, :] = hi4[:, k1]
    W1dr = np.ascontiguousarray(W1dr.reshape(TWO_N, 2, 128, 8192))

    shared = {
        "W0": W0f,
        "W1": W1dr,
        "W2": np.ascontiguousarray(np.asarray(W2, np.float32)
                                   .astype(np.float16)),
        "W3": np.ascontiguousarray(np.asarray(W3, np.float32)
                                   .astype(np.float16)),
        "b0": np.ascontiguousarray(np.asarray(b0, np.float32)),
        "b1": np.ascontiguousarray(np.asarray(b1, np.float32)),
        "b2": np.ascontiguousarray(np.asarray(b2, np.float32)),
        "b3": np.ascontiguousarray(np.asarray(b3, np.float32)),
    }
    in_maps = []
    for c in range(N_CORES):
        m = dict(shared)
        m["x"] = np.ascontiguousarray(x16[B_LOC * c:B_LOC * (c + 1), :])
        in_maps.append(m)
    return in_maps


def kernel(x, W0, b0, W1, b1, W2, b2, W3, b3):
    global _NC
    if _NC is None:
        _NC = _build_program()
    in_maps = build_in_maps(x, W0, b0, W1, b1, W2, b2, W3, b3)
    res = run_bass_kernel_spmd(_NC, in_maps, list(range(N_CORES)))
    return np.concatenate([res.results[c]["y"] for c in range(N_CORES)],
                          axis=0)
